# revision 1
# baseline (speedup 1.0000x reference)
"""Trainium2 Bass kernel for nn_Conv4dNet: 6x conv4d(3^4) + BN4d + ReLU.

Strategy: spatial shard over outermost spatial dim 'a' across 8 NeuronCores
(7 active, 2 planes each; core 7 runs dummy data for SPMD uniformity).
One SPMD launch per conv layer; host (numpy) does BN stats + BN/ReLU + halo
re-slicing between launches (exact math, negligible cost vs conv).

Device conv scheme per layer:
  - padded-plane layout: each (b,c,d) cube padded to 16x16x16 = 4096 cols,
    data at +1 offsets, zero pads -> all 3^4 tap shifts are affine col offsets.
  - matmul: stationary = W [K=Ci-chunk, M=3*Cog (dc folded into M)],
    moving = input slab [K, N<=512] with col shift (db-1)*256+(dd-1),
    accumulate over taps (da,db,dd)xKchunks in PSUM (fp32r = full-rate fp32).
  - epilogue: out[co,n] = p[dc0,n-16] + p[dc1,n] + p[dc2,n+16] (2 DVE adds).
"""
import sys
import os

sys.path.insert(0, "/opt/trn_rl_repo")
import numpy as np

import concourse.bass as bass
import concourse.mybir as mybir
from concourse import tile
from concourse.bass_utils import run_bass_kernel_spmd

DT = mybir.dt
EPS = 1e-5
D = 14
PLANE = 4096  # 16*16*16
GUARD = 288
NCORES = 8
NACT = 7  # cores 0..6 own 2 planes each
CHANS = [(1, 40), (40, 80), (80, 160), (160, 80), (80, 40), (40, 1)]

# psum window layout per output plane (plane cols):
#   half A: matmul windows [256,2304) as 4x512, epilogue out [272,2288)
#   half B: windows [2272,3840) as 512,512,512,32, epilogue out [2288,3824)
HALVES = [
    (256, [512, 512, 512, 512], 16, 2032),   # (col0, window sizes, out_lo, out_hi) rel to col0
    (2272, [512, 512, 512, 32], 16, 1552),
]

_CACHE = {}


def _chunks(n, sz):
    out = []
    i = 0
    while i < n:
        out.append((i, min(sz, n - i)))
        i += sz
    return out


def _layer_plan(ci, co):
    kchunks = _chunks(ci, 128)
    # M = 3*cog <= 128 -> cog <= 42; use 40 for clean splits
    cog = min(co, 40)
    mchunks = _chunks(co, cog)
    return kchunks, mchunks


def pack_weights(w):
    """w: [Co, Ci, 3,3,3,3] -> per-Mchunk stationary [128, ntap*nk*3*cogmax]."""
    co, ci = w.shape[0], w.shape[1]
    kchunks, mchunks = _layer_plan(ci, co)
    packs = []
    for m0, mlen in mchunks:
        blocks = []
        for da in range(3):
            for db in range(3):
                for dd in range(3):
                    for k0, klen in kchunks:
                        st = np.zeros((128, 3 * mlen), dtype=np.float32)
                        for dc in range(3):
                            # rows=ci, cols = dc*mlen + co_local
                            st[:klen, dc * mlen:(dc + 1) * mlen] = (
                                w[m0:m0 + mlen, k0:k0 + klen, da, db, dc, dd].T
                            )
                        blocks.append(st)
                    # pad so every Mchunk has same block count? (ragged ok, per-layer fixed)
        packs.append(np.concatenate(blocks, axis=1))
    return packs  # list per mchunk: [128, nblocks*3*mlen]


def build_conv_nc(ci, co):
    """One SPMD conv layer kernel: in [Ci, 4*PLANE+2G] -> out [Co, 2*PLANE]."""
    nc = bass.Bass("TRN2")
    kchunks, mchunks = _layer_plan(ci, co)
    ntap = 27
    in_cols = 2 * GUARD + 4 * PLANE
    xin = nc.dram_tensor("xin", [ci, in_cols], DT.float32r, kind="ExternalInput")
    wts = [
        nc.dram_tensor(f"w_m{mi}", [128, ntap * len(kchunks) * 3 * mlen],
                       DT.float32r, kind="ExternalInput")
        for mi, (m0, mlen) in enumerate(mchunks)
    ]
    yout = nc.dram_tensor("yout", [co, 2 * PLANE], DT.float32, kind="ExternalOutput")

    with tile.TileContext(nc) as tc:
        with tc.tile_pool(name="xin_p", bufs=1) as xp, \
             tc.tile_pool(name="out_p", bufs=1) as op, \
             tc.tile_pool(name="w_p", bufs=1) as wp, \
             tc.tile_pool(name="tmp_p", bufs=2) as tp, \
             tc.tile_pool(name="ps_p", bufs=2, space="PSUM") as pp:
            # input slab tiles per Kchunk
            xts = []
            for k0, klen in kchunks:
                xt = xp.tile([klen, in_cols], DT.float32r, name=f"x_{k0}")
                nc.gpsimd.dma_start(xt[:, :], xin[k0:k0 + klen, :])
                xts.append(xt)
            # output tiles per <=128-channel group
            octs = _chunks(co, 128)
            outs = [op.tile([cl, 2 * PLANE], DT.float32, name=f"o_{c0}")
                    for c0, cl in octs]

            def out_slice(c0, clen, pq, lo, hi):
                # rows c0:c0+clen of output, plane pq (0/1), cols lo:hi
                for i, (g0, gl) in enumerate(octs):
                    if g0 <= c0 < g0 + gl:
                        return outs[i][c0 - g0:c0 - g0 + clen,
                                       pq * PLANE + lo:pq * PLANE + hi]
                raise AssertionError

            for mi, (m0, mlen) in enumerate(mchunks):
                wt = wp.tile([128, ntap * len(kchunks) * 3 * mlen],
                             DT.float32r, name="wt", tag="wt")
                nc.gpsimd.dma_start(wt[:, :], wts[mi][:, :])
                mw = 3 * mlen
                for pq in range(2):          # output plane (slots 1,2)
                    slot = 1 + pq
                    for (c0h, wins, olo, ohi) in HALVES:
                        pt = pp.tile([128, 2048], DT.float32, name="ps", tag="ps")
                        nmm = ntap * len(kchunks) * len(wins)
                        imm = 0
                        blk = 0
                        for da in range(3):
                            for db in range(3):
                                for dd in range(3):
                                    for kci, (k0, klen) in enumerate(kchunks):
                                        woff = blk * mw
                                        st = wt[0:klen, woff:woff + mw]
                                        base = (GUARD + (slot + da - 1) * PLANE
                                                + c0h + (db - 1) * 256 + (dd - 1))
                                        woffp = 0
                                        for wn in wins:
                                            mv = xts[kci][0:klen,
                                                          base + woffp:base + woffp + wn]
                                            nc.tensor.matmul(
                                                pt[0:mw, woffp:woffp + wn],
                                                st,
                                                mv,
                                                start=(imm == 0),
                                                stop=(imm == nmm - 1),
                                            )
                                            imm += 1
                                            woffp += wn
                                        blk += 1
                        # epilogue: out = p[dc0]@(n-16) + p[dc1]@n + p[dc2]@(n+16)
                        tt = tp.tile([mlen, 2048], DT.float32, name="tt", tag="tt")
                        n0, n1 = olo, ohi
                        nc.vector.tensor_add(
                            tt[:, n0:n1],
                            pt[0:mlen, n0 - 16:n1 - 16],
                            pt[mlen:2 * mlen, n0:n1],
                        )
                        nc.vector.tensor_add(
                            out_slice(m0, mlen, pq, c0h + n0, c0h + n1),
                            tt[:, n0:n1],
                            pt[2 * mlen:3 * mlen, n0 + 16:n1 + 16],
                        )
            for i, (g0, gl) in enumerate(octs):
                nc.gpsimd.dma_start(yout[g0:g0 + gl, :], outs[i][:, :])
    return nc


def _get_nc(ci, co):
    if (ci, co) not in _CACHE:
        _CACHE[(ci, co)] = build_conv_nc(ci, co)
    return _CACHE[(ci, co)]


def _pad_volume(h):
    """h: [C, 14,14,14,14] -> padded [C, 16, PLANE] with +1 offsets, zero pads."""
    c = h.shape[0]
    hp = np.zeros((c, 16, 16, 16, 16), dtype=np.float32)
    hp[:, 1:15, 1:15, 1:15, 1:15] = h
    return hp.reshape(c, 16, PLANE)


def _conv_layer_on_device(hp, wpacks, ci, co):
    """hp: padded [Ci, 16, PLANE]. Returns conv out [Co, 14,14,14,14]."""
    nc = _get_nc(ci, co)
    in_cols = 2 * GUARD + 4 * PLANE
    in_maps = []
    for cidx in range(NCORES):
        cc = min(cidx, NACT - 1)  # core 7 duplicates core 6 (output ignored)
        slab = np.zeros((ci, in_cols), dtype=np.float32)
        # slots 0..3 = padded planes 2c .. 2c+3
        slab[:, GUARD:GUARD + 4 * PLANE] = hp[:, 2 * cc:2 * cc + 4, :].reshape(ci, -1)
        im = {"xin": slab}
        for mi, wpk in enumerate(wpacks):
            im[f"w_m{mi}"] = wpk
        in_maps.append(im)
    res = run_bass_kernel_spmd(nc, in_maps, core_ids=list(range(NCORES)))
    out = np.zeros((co, D, 16, 16, 16), dtype=np.float32)
    for cc in range(NACT):
        y = res.results[cc]["yout"].reshape(co, 2, 16, 16, 16)
        out[:, 2 * cc:2 * cc + 2] = y
    return out[:, :, 1:15, 1:15, 1:15]


def _conv4d_np(x, w):
    ci, a, b, c, d = x.shape
    co = w.shape[0]
    xp = np.zeros((ci, a + 2, b + 2, c + 2, d + 2), dtype=np.float64)
    xp[:, 1:-1, 1:-1, 1:-1, 1:-1] = x
    out = np.zeros((co, a, b, c, d), dtype=np.float64)
    for ta in range(3):
        for tb in range(3):
            for tc_ in range(3):
                for td in range(3):
                    seg = xp[:, ta:ta + a, tb:tb + b, tc_:tc_ + c, td:td + d]
                    out += np.einsum("oi,ixyzw->oxyzw",
                                     w[:, :, ta, tb, tc_, td].astype(np.float64),
                                     seg, optimize=True)
    return out.astype(np.float32)


_DEVICE_OK = [True]


def _conv_dispatch(hp_or_h, w, wpacks, ci, co):
    if _DEVICE_OK[0]:
        try:
            return _conv_layer_on_device(_pad_volume(hp_or_h), wpacks, ci, co)
        except Exception as e:
            import traceback; traceback.print_exc()
            _DEVICE_OK[0] = False
    return _conv4d_np(hp_or_h, w)


def kernel(**inputs):
    x = np.asarray(inputs["x"], dtype=np.float32).reshape(1, D, D, D, D)
    h = x
    for li, (ci, co) in enumerate(CHANS, start=1):
        w = np.asarray(inputs[f"w{li}"], dtype=np.float32)
        wpacks = pack_weights(w)
        hconv = _conv_dispatch(h, w, wpacks, ci, co)  # [co,14^4]
        if li < 6:
            g = np.asarray(inputs[f"g{li}"], dtype=np.float32)
            b = np.asarray(inputs[f"b{li}"], dtype=np.float32)
            mean = hconv.mean(axis=(1, 2, 3, 4), keepdims=True)
            var = hconv.var(axis=(1, 2, 3, 4), keepdims=True)
            h = (hconv - mean) / np.sqrt(var + EPS) * g.reshape(-1, 1, 1, 1, 1) \
                + b.reshape(-1, 1, 1, 1, 1)
            h = np.maximum(h, 0.0)
        else:
            b6 = np.asarray(inputs["b6"], dtype=np.float32)
            h = np.maximum(hconv + b6.reshape(-1, 1, 1, 1, 1), 0.0)
    return h.reshape(1, 1, D, D, D, D).astype(np.float32)



# revision 7
# speedup vs baseline: 1.0275x; 1.0275x over previous
"""Trainium2 fused Bass kernel for nn_Conv4dNet: 6x conv4d(3^4) + BN4d + ReLU.

Single SPMD launch, one NEFF, 8 NeuronCores:
  - spatial shard over outermost dim 'a' (14 planes): cores 0..6 own 2
    output planes each; core 7 runs the same program on garbage data and
    its BN-stats contribution is zeroed via an uploaded flag.
  - weights are uploaded column-sharded (1/8th per core) and AllGathered
    on device into DRAM once.
  - per conv layer: taps-as-matmuls accumulate in PSUM (fp32r), DVE
    epilogue folds the 3 dc-shifted copies, pad columns re-zeroed by
    strided memsets, local BN stats reduced on-chip, AllReduce'd (tiny),
    activations AllGathered through DRAM, halo slab rebuilt with
    dynamic-offset DMAs (per-core row table, OOB rows skip), BN+ReLU
    applied as one scalar-engine activation.

Compiled NEFFs are cached under ~/.neuron-compile-cache, so repeat runs
skip neuronx-cc entirely.
"""
import sys

sys.path.insert(0, "/opt/trn_rl_repo")
import numpy as np

import concourse.bass as bass
import concourse.mybir as mybir
from concourse import tile
from concourse.bass_utils import run_bass_kernel_spmd

DT = mybir.dt
AF = mybir.ActivationFunctionType
ALU = mybir.AluOpType
AX = mybir.AxisListType

EPS = 1e-5
D = 14
PLANE = 4096          # 16*16*16 padded cube per 'a'-plane
GUARD = 64
SLABW = 2 * GUARD + 4 * PLANE
N_VOX = float(D ** 4)
SENT = 1 << 27        # OOB row sentinel -> halo DMA skipped
NCORES = 8
NACT = 7
CHANS = [(1, 40), (40, 80), (80, 160), (160, 80), (80, 40), (40, 1)]

# (plane col base, matmul window sizes, epilogue n0, epilogue n1)
HALVES = [
    (256, [512, 512, 512, 512], 16, 2032),
    (2272, [512, 512, 512, 32], 16, 1552),
]

PACKW = 3240          # uniform wt tile cols: 27 taps * 3 * mlen(<=40)


def _kchunks(ci):
    if ci == 160:
        return [(0, 80), (80, 80)]
    return [(0, ci)]


def _mchunks(co):
    if co == 1:
        return [(0, 1)]
    return [(m0, 40) for m0 in range(0, co, 40)]


def _cogroups(co):
    if co == 160:
        return [(0, 80), (80, 80)]
    return [(0, co)]


def _wall_layout():
    """Column layout of the concatenated weight packs: (l, mi, ki) -> (col0, ncols)."""
    lay = {}
    off = 0
    for li, (ci, co) in enumerate(CHANS):
        kch = _kchunks(ci)
        mch = _mchunks(co)
        for mi, (m0, mlen) in enumerate(mch):
            for ki in range(len(kch)):
                ncols = 27 * 3 * mlen
                lay[(li, mi, ki)] = (off, ncols)
                off += ncols
    tot = off
    shard = -(-tot // NCORES)
    shard = ((shard + 127) // 128) * 128   # keep shards 512B-aligned rows
    return lay, tot, shard


WALL_LAYOUT, WALL_TOT, WALL_SHARD = _wall_layout()

# gb column map: (layer idx 0..4, group idx) -> (g col, b col); b6 at col 12
GB_COLS = {(0, 0): (0, 1), (1, 0): (2, 3), (2, 0): (4, 5), (2, 1): (6, 7),
           (3, 0): (8, 9), (4, 0): (10, 11)}
B6_COL = 12

# halo table: single int per core = first gathered plane row (2c; pads at 0/17)
HALO_LEN = 1


# ---------------------------------------------------------------------------
# multi-wait splitting post-pass (this walrus build accepts only one sync
# wait command per instruction; move extras to single-wait carriers)
# ---------------------------------------------------------------------------
_WSPLIT = [0]


def _split_multi_waits(nc):
    for f in nc.m.functions:
        for bb in f.blocks:
            insts = bb.instructions
            i = 0
            while i < len(insts):
                ins = insts[i]
                si = ins.sync_info
                if si is None:
                    i += 1
                    continue
                waits = list(si.on_wait)
                keep = 0 if isinstance(ins, mybir.InstDrain) else 1
                if len(waits) <= keep:
                    i += 1
                    continue
                moved = waits[: len(waits) - keep]
                kept = waits[len(waits) - keep:]
                carriers = []
                for w in moved:
                    _WSPLIT[0] += 1
                    nop = mybir.InstEventSemaphore(
                        name=f"wsplit-{_WSPLIT[0]}", ins=[], outs=[]
                    )
                    nop.engine = ins.engine
                    nop.sync_info = mybir.SyncInfo(on_wait=[w], on_update=[])
                    carriers.append(nop)
                ins.sync_info = mybir.SyncInfo(
                    on_wait=kept, on_update=list(si.on_update)
                )
                for k, c in enumerate(carriers):
                    insts.insert(i + k, c)
                i += len(carriers) + 1


# ---------------------------------------------------------------------------
# device kernel build
# ---------------------------------------------------------------------------
def _zero_pad_stripes(nc, ap_plane_rows):
    """Memset the pad columns (b,c,d in {0,15}) of plane-shaped AP(s).

    ap_plane_rows: AP of shape [rows, nplanes, 4096] (3D) or [rows, 4096].
    Uses strided sub-APs so each stripe family is one memset.
    """
    a = ap_plane_rows
    if a.ndim == 2:
        a = a.unsqueeze(1)
    # b = 0 / 15: cols [0,256) and [3840,4096)
    nc.vector.memset(a[:, :, 0:256], 0.0)
    nc.vector.memset(a[:, :, 3840:4096], 0.0)
    # c = 0 / 15: for each b block: cols b*256 + [0,16) / [240,256)
    r = a.rearrange("p n (b c) -> p n b c", c=256)
    nc.vector.memset(r[:, :, :, 0:16], 0.0)
    nc.vector.memset(r[:, :, :, 240:256], 0.0)
    # d = 0 / 15: every 16th col
    s = a.rearrange("p n (bc d) -> p n bc d", d=16)
    nc.vector.memset(s[:, :, :, 0:1], 0.0)
    nc.vector.memset(s[:, :, :, 15:16], 0.0)


def build_fused_nc():
    nc = bass.Bass("TRN2")
    f32 = DT.float32

    xslab = nc.dram_tensor("xslab", [1, SLABW], f32, kind="ExternalInput")
    wsh = nc.dram_tensor("wsh", [128, WALL_SHARD], f32, kind="ExternalInput")
    halo_tab = nc.dram_tensor("halo_tab", [1, HALO_LEN], DT.int32,
                              kind="ExternalInput")
    slotflag = nc.dram_tensor("slotflag", [128, 4], f32, kind="ExternalInput")
    actflag = nc.dram_tensor("actflag", [128, 1], f32, kind="ExternalInput")
    gb = nc.dram_tensor("gb", [128, 13], f32, kind="ExternalInput")
    yout = nc.dram_tensor("yout", [1, 2 * PLANE], f32, kind="ExternalOutput")

    wall_g = nc.dram_tensor("wall_g", [NCORES * 128, WALL_SHARD], f32,
                            kind="Internal", addr_space="Shared")
    ccin = {}
    ccout = {}
    stin = {}
    stout = {}
    for li in range(5):
        co = CHANS[li][1]
        ccin[li] = nc.dram_tensor(f"ccin{li}", [2, co, PLANE], f32,
                                  kind="Internal")
        ccout[li] = nc.dram_tensor(f"ccout{li}", [2 * NCORES + 2, co, PLANE],
                                   f32, kind="Internal", addr_space="Shared")
        stin[li] = nc.dram_tensor(f"stin{li}", [co, 2], f32, kind="Internal")
        stout[li] = nc.dram_tensor(f"stout{li}", [co, 2], f32,
                                   kind="Internal", addr_space="Shared")

    groups8 = [list(range(NCORES))]

    with tile.TileContext(nc) as tc:
        with tc.tile_pool(name="slab_p", bufs=1) as slab_p, \
             tc.tile_pool(name="wt_p", bufs=1) as wt_p, \
             tc.tile_pool(name="out_p", bufs=1) as out_p, \
             tc.tile_pool(name="sm_p", bufs=1) as sm_p, \
             tc.tile_pool(name="ps_p", bufs=2, space="PSUM") as ps_p:

            # ---- weights AllGather (sharded upload -> full table in DRAM)
            nc.gpsimd.collective_compute(
                "AllGather", ALU.bypass, replica_groups=groups8,
                ins=[wsh[:, :]], outs=[wall_g[:, :]],
            )

            # ---- persistent small tiles
            gbt = sm_p.tile([128, 13], f32, name="gbt")
            nc.gpsimd.dma_start(gbt[:, :], gb[:, :])
            aft = sm_p.tile([128, 1], f32, name="aft")
            nc.gpsimd.dma_start(aft[:, :], actflag[:, :])
            sft = sm_p.tile([128, 4], f32, name="sft")
            nc.gpsimd.dma_start(sft[:, :], slotflag[:, :])
            htt = sm_p.tile([1, HALO_LEN], DT.int32, name="htt")
            nc.gpsimd.dma_start(htt[:, :], halo_tab[:, :])
            zt = sm_p.tile([1, 512], f32, name="zt")
            nc.vector.memset(zt[:, :], 0.0)

            # ---- initial slab (layer 1 input, 1 channel)
            slabs = [slab_p.tile([128, SLABW], f32, name="slab_g0",
                                 tag="slab_g0")]
            nc.gpsimd.dma_start(slabs[0][0:1, :], xslab[:, :])

            def load_wt(li, mi, ki, klen):
                t = wt_p.tile([128, PACKW], f32, name=f"wt{li}_{mi}_{ki}",
                              tag=f"wt_k{ki}")
                col0, ncols = WALL_LAYOUT[(li, mi, ki)]
                dst = 0
                c = col0
                end = col0 + ncols
                while c < end:
                    s = c // WALL_SHARD
                    take = min(end - c, (s + 1) * WALL_SHARD - c)
                    nc.gpsimd.dma_start(
                        t[0:klen, dst:dst + take],
                        wall_g[s * 128:s * 128 + klen,
                               c - s * WALL_SHARD:c - s * WALL_SHARD + take],
                    )
                    c += take
                    dst += take
                return t

            hreg_cm = nc.gpsimd.register("hreg")
            hreg = hreg_cm.__enter__()
            nc.gpsimd.reg_load(hreg, htt[0:1, 0:1])
            vplane = nc.gpsimd.snap(hreg)

            for li, (ci, co) in enumerate(CHANS):
                kch = _kchunks(ci)
                mch = _mchunks(co)
                cog = _cogroups(co)
                mlen = mch[0][1]
                mw = 3 * mlen
                last = len(kch) * 27 - 1

                # output plane tiles per cogroup (one plane at a time)
                st_tiles = {}
                if li < 5:
                    for gi, (r0, gl) in enumerate(cog):
                        st = sm_p.tile([128, 4], f32, name=f"st{li}_{gi}",
                                       tag=f"st_g{gi}")
                        nc.vector.memset(st[0:gl, :], 0.0)
                        st_tiles[gi] = st

                for q in (1, 2):          # output plane slot in slab
                    ptiles = {}
                    for gi, (r0, gl) in enumerate(cog):
                        pt = out_p.tile([128, PLANE], f32,
                                        name=f"o{li}_{q}_{gi}",
                                        tag=f"out_g{gi}")
                        ptiles[gi] = pt
                    for mi, (m0, _) in enumerate(mch):
                        wts = [load_wt(li, mi, ki, kl)
                               for ki, (k0, kl) in enumerate(kch)]
                        gi = m0 // 80 if co == 160 else 0
                        lr = m0 - cog[gi][0]    # local row in group tile
                        pt = ptiles[gi]
                        for (c0h, wins, n0, n1) in HALVES:
                            ps = ps_p.tile([128, 2048], f32, name="ps",
                                           tag="ps")
                            tap = 0
                            for da in range(3):
                                for db in range(3):
                                    for dd in range(3):
                                        blk = (da * 3 + db) * 3 + dd
                                        for ki, (k0, kl) in enumerate(kch):
                                            stt = wts[ki][0:kl,
                                                          blk * mw:blk * mw + mw]
                                            base = (GUARD
                                                    + (q + da - 1) * PLANE
                                                    + c0h + (db - 1) * 256
                                                    + (dd - 1))
                                            woffp = 0
                                            for wn in wins:
                                                mv = slabs[ki][0:kl,
                                                               base + woffp:
                                                               base + woffp + wn]
                                                nc.tensor.matmul(
                                                    ps[0:mw, woffp:woffp + wn],
                                                    stt.bitcast(DT.float32r),
                                                    mv.bitcast(DT.float32r),
                                                    start=(tap == 0),
                                                    stop=(tap == last),
                                                )
                                                woffp += wn
                                            tap += 1
                            # epilogue: out[n] = p0[n-16] + p1[n] + p2[n+16]
                            tt = sm_p.tile([128, 2048], f32, name="tt",
                                           tag="tt")
                            nc.vector.tensor_add(
                                tt[0:mlen, n0:n1],
                                ps[0:mlen, n0 - 16:n1 - 16],
                                ps[mlen:2 * mlen, n0:n1],
                            )
                            nc.vector.tensor_add(
                                pt[lr:lr + mlen, c0h + n0:c0h + n1],
                                tt[0:mlen, n0:n1],
                                ps[2 * mlen:3 * mlen, n0 + 16:n1 + 16],
                            )
                    # per-plane post-processing
                    for gi, (r0, gl) in enumerate(cog):
                        pt = ptiles[gi]
                        if li < 5:
                            _zero_pad_stripes(nc, pt[0:gl, :])
                            st = st_tiles[gi]
                            nc.vector.tensor_reduce(
                                st[0:gl, 2:3], pt[0:gl, :], AX.X, ALU.add)
                            nc.vector.tensor_add(
                                st[0:gl, 0:1], st[0:gl, 0:1], st[0:gl, 2:3])
                            nc.gpsimd.dma_start(
                                ccin[li][q - 1:q, r0:r0 + gl, :],
                                pt[0:gl, :])
                            # in-place square (after the DMA read) + accum
                            nc.scalar.activation(
                                pt[0:gl, :], pt[0:gl, :], AF.Square,
                                accum_out=st[0:gl, 3:4])
                            nc.vector.tensor_add(
                                st[0:gl, 1:2], st[0:gl, 1:2], st[0:gl, 3:4])
                        else:
                            # final layer: + b6, relu, ship out
                            nc.scalar.activation(
                                pt[0:1, :], pt[0:1, :], AF.Relu,
                                bias=gbt[0:1, B6_COL:B6_COL + 1])
                            nc.gpsimd.dma_start(
                                yout[0:1, (q - 1) * PLANE:q * PLANE],
                                pt[0:1, :])

                if li == 5:
                    break

                # ---- stats collective (tiny) then activation AllGather
                for gi, (r0, gl) in enumerate(cog):
                    st = st_tiles[gi]
                    nc.vector.tensor_scalar_mul(
                        st[0:gl, 0:2], st[0:gl, 0:2], aft[0:gl, 0:1])
                    nc.gpsimd.dma_start(stin[li][r0:r0 + gl, :],
                                        st[0:gl, 0:2])
                nc.gpsimd.collective_compute(
                    "AllReduce", ALU.add, replica_groups=groups8,
                    ins=[stin[li][:, :]], outs=[stout[li][:, :]],
                )
                for pr in (0, 2 * NCORES + 1):
                    nc.gpsimd.dma_start(
                        ccout[li][pr:pr + 1, :, :].rearrange(
                            "a b (x y) -> a (b x) y", y=512),
                        zt[0:1, :].unsqueeze(1).to_broadcast(
                            [1, co * 8, 512]))
                nc.gpsimd.collective_compute(
                    "AllGather", ALU.bypass, replica_groups=groups8,
                    ins=[ccin[li][:, :, :]],
                    outs=[ccout[li][1:2 * NCORES + 1, :, :]],
                )

                # ---- BN scale/shift from reduced stats
                bn_tiles = {}
                for gi, (r0, gl) in enumerate(cog):
                    sg = sm_p.tile([128, 2], f32, name=f"sg{li}_{gi}",
                                   tag=f"sg_g{gi}")
                    nc.gpsimd.dma_start(sg[0:gl, :], stout[li][r0:r0 + gl, :])
                    t = sm_p.tile([128, 8], f32, name=f"bnt{li}_{gi}",
                                  tag=f"bnt_g{gi}")
                    gc, bc = GB_COLS[(li, gi)]
                    nc.vector.tensor_scalar_mul(
                        t[0:gl, 0:2], sg[0:gl, 0:2], 1.0 / N_VOX)
                    nc.vector.tensor_mul(
                        t[0:gl, 2:3], t[0:gl, 0:1], t[0:gl, 0:1])
                    nc.vector.tensor_sub(
                        t[0:gl, 3:4], t[0:gl, 1:2], t[0:gl, 2:3])
                    nc.vector.tensor_scalar_add(
                        t[0:gl, 3:4], t[0:gl, 3:4], EPS)
                    nc.scalar.sqrt(t[0:gl, 4:5], t[0:gl, 3:4])
                    nc.vector.reciprocal(t[0:gl, 5:6], t[0:gl, 4:5])
                    nc.vector.tensor_mul(
                        t[0:gl, 6:7], t[0:gl, 5:6], gbt[0:gl, gc:gc + 1])
                    nc.vector.tensor_mul(
                        t[0:gl, 2:3], t[0:gl, 0:1], t[0:gl, 6:7])
                    nc.vector.tensor_sub(
                        t[0:gl, 7:8], gbt[0:gl, bc:bc + 1], t[0:gl, 2:3])
                    bn_tiles[gi] = t

                # ---- rebuild slab for next layer from gathered acts
                nci = CHANS[li + 1][0]
                nkch = _kchunks(nci)
                new_slabs = []
                for gi, (r0, gl) in enumerate(nkch):
                    sl = slab_p.tile([128, SLABW], f32, name=f"slab{li}_{gi}",
                                     tag=f"slab_g{gi}")
                    # planes 2c-1 .. 2c+2 (pad planes at rows 0 / 17)
                    dst = sl[0:gl, GUARD:GUARD + 4 * PLANE].rearrange(
                        "p (s c) -> p s c", c=PLANE)
                    srcp = ccout[li][bass.ds(vplane, 4), r0:r0 + gl,
                                     :].transpose([1, 0, 2])
                    nc.gpsimd.dma_start(dst, srcp)
                    # BN + ReLU in place over all 4 slots
                    t = bn_tiles[gi]
                    nc.scalar.activation(
                        sl[0:gl, GUARD:GUARD + 4 * PLANE],
                        sl[0:gl, GUARD:GUARD + 4 * PLANE],
                        AF.Relu, bias=t[0:gl, 7:8], scale=t[0:gl, 6:7])
                    # re-zero pads: guards, pad cols, invalid slots
                    nc.vector.memset(sl[0:gl, 0:GUARD], 0.0)
                    nc.vector.memset(
                        sl[0:gl, GUARD + 4 * PLANE:SLABW], 0.0)
                    planes4 = sl[0:gl, GUARD:GUARD + 4 * PLANE].rearrange(
                        "p (n c) -> p n c", c=PLANE)
                    _zero_pad_stripes(nc, planes4)
                    for s in range(4):
                        nc.vector.tensor_scalar_mul(
                            sl[0:gl, GUARD + s * PLANE:GUARD + (s + 1) * PLANE],
                            sl[0:gl, GUARD + s * PLANE:GUARD + (s + 1) * PLANE],
                            sft[0:gl, s:s + 1])
                    new_slabs.append(sl)
                slabs = new_slabs

            hreg_cm.__exit__(None, None, None)

    _split_multi_waits(nc)
    return nc


_NC_CACHE = []


def _get_nc():
    if not _NC_CACHE:
        _NC_CACHE.append(build_fused_nc())
    return _NC_CACHE[0]


# ---------------------------------------------------------------------------
# host side
# ---------------------------------------------------------------------------
def _pack_wall(ws):
    """ws: list of 6 arrays [co, ci, 3,3,3,3] -> [128, NCORES*WALL_SHARD]."""
    wall = np.zeros((128, NCORES * WALL_SHARD), dtype=np.float32)
    for li, (ci, co) in enumerate(CHANS):
        w = ws[li]
        kch = _kchunks(ci)
        mch = _mchunks(co)
        for mi, (m0, mlen) in enumerate(mch):
            for ki, (k0, kl) in enumerate(kch):
                col0, ncols = WALL_LAYOUT[(li, mi, ki)]
                blkw = 3 * mlen
                for da in range(3):
                    for db in range(3):
                        for dd in range(3):
                            blk = (da * 3 + db) * 3 + dd
                            for dc in range(3):
                                # [kl, mlen] = w[m0:m0+mlen, k0:k0+kl, ...].T
                                wall[0:kl,
                                     col0 + blk * blkw + dc * mlen:
                                     col0 + blk * blkw + (dc + 1) * mlen] = \
                                    w[m0:m0 + mlen, k0:k0 + kl,
                                      da, db, dc, dd].T
    return wall


def _pack_xslab(x, c):
    """x: [14,14,14,14] -> per-core slab [1, SLABW] (planes 2c-1 .. 2c+2)."""
    slab = np.zeros((1, SLABW), dtype=np.float32)
    for s in range(4):
        p = 2 * c - 1 + s
        if 0 <= p < D:
            cube = np.zeros((16, 16, 16), dtype=np.float32)
            cube[1:15, 1:15, 1:15] = x[p]
            slab[0, GUARD + s * PLANE:GUARD + (s + 1) * PLANE] = cube.reshape(-1)
    return slab


def _make_inmaps(inputs):
    x = np.asarray(inputs["x"], dtype=np.float32).reshape(D, D, D, D)
    ws = [np.asarray(inputs[f"w{i}"], dtype=np.float32) for i in range(1, 7)]
    wall = _pack_wall(ws)

    gb = np.zeros((128, 13), dtype=np.float32)
    for li in range(5):
        co = CHANS[li][1]
        g = np.asarray(inputs[f"g{li + 1}"], dtype=np.float32)
        b = np.asarray(inputs[f"b{li + 1}"], dtype=np.float32)
        for gi, (r0, gl) in enumerate(_cogroups(co)):
            gc, bc = GB_COLS[(li, gi)]
            gb[0:gl, gc] = g[r0:r0 + gl]
            gb[0:gl, bc] = b[r0:r0 + gl]
    gb[0:1, B6_COL] = np.asarray(inputs["b6"], dtype=np.float32)

    in_maps = []
    for c in range(NCORES):
        tab = np.array([[2 * c]], dtype=np.int32)
        sflag = np.ones((128, 4), dtype=np.float32)
        if c == 0:
            sflag[:, 0] = 0.0
        if c >= 6:
            sflag[:, 3] = 0.0
        aflag = np.full((128, 1), 1.0 if c < NACT else 0.0, dtype=np.float32)
        in_maps.append({
            "xslab": _pack_xslab(x, c),
            "wsh": np.ascontiguousarray(
                wall[:, c * WALL_SHARD:(c + 1) * WALL_SHARD]),
            "halo_tab": tab,
            "slotflag": sflag,
            "actflag": aflag,
            "gb": gb,
        })
    return in_maps


def _run_device(inputs):
    nc = _get_nc()
    in_maps = _make_inmaps(inputs)
    res = run_bass_kernel_spmd(nc, in_maps, core_ids=list(range(NCORES)))
    out = np.zeros((D, D, D, D), dtype=np.float32)
    for c in range(NACT):
        y = res.results[c]["yout"].reshape(2, 16, 16, 16)
        for k in range(2):
            out[2 * c + k] = y[k][1:15, 1:15, 1:15]
    return out.reshape(1, 1, D, D, D, D)


# ---------------------------------------------------------------------------
# numpy fallback (exact math, used only if the device path throws)
# ---------------------------------------------------------------------------
def _conv4d_np(x, w):
    ci, a, b, c, d = x.shape
    xp = np.zeros((ci, a + 2, b + 2, c + 2, d + 2), dtype=np.float64)
    xp[:, 1:-1, 1:-1, 1:-1, 1:-1] = x
    co = w.shape[0]
    out = np.zeros((co, a, b, c, d), dtype=np.float64)
    for ta in range(3):
        for tb in range(3):
            for tc_ in range(3):
                for td in range(3):
                    seg = xp[:, ta:ta + a, tb:tb + b, tc_:tc_ + c, td:td + d]
                    out += np.einsum(
                        "oi,ixyzw->oxyzw",
                        w[:, :, ta, tb, tc_, td].astype(np.float64), seg,
                        optimize=True)
    return out


def _run_numpy(inputs):
    h = np.asarray(inputs["x"], dtype=np.float64).reshape(1, D, D, D, D)
    for li in range(1, 7):
        w = np.asarray(inputs[f"w{li}"], dtype=np.float64)
        h = _conv4d_np(h, w)
        if li < 6:
            g = np.asarray(inputs[f"g{li}"], dtype=np.float64)
            b = np.asarray(inputs[f"b{li}"], dtype=np.float64)
            mean = h.mean(axis=(1, 2, 3, 4), keepdims=True)
            var = h.var(axis=(1, 2, 3, 4), keepdims=True)
            h = (h - mean) / np.sqrt(var + EPS) * g.reshape(-1, 1, 1, 1, 1) \
                + b.reshape(-1, 1, 1, 1, 1)
            h = np.maximum(h, 0.0)
        else:
            b6 = np.asarray(inputs["b6"], dtype=np.float64)
            h = np.maximum(h + b6.reshape(-1, 1, 1, 1, 1), 0.0)
    return h.reshape(1, 1, D, D, D, D).astype(np.float32)


def kernel(**inputs):
    try:
        return _run_device(inputs)
    except Exception:
        import traceback
        traceback.print_exc()
        return _run_numpy(inputs)
